# revision 9
# baseline (speedup 1.0000x reference)
"""Trainium2 Bass kernel for BaseViTSelfAttention (cross/self attention, 16 heads).

Computation (per batch element b):
    q = hidden @ Wq.T            [1024, 1024]
    ctx = concat(hidden, context)  [1280, 1024]
    k = ctx @ Wk.T; v = ctx @ Wv.T
    out = softmax(q_h @ k_h.T / 8) @ v_h   per 64-dim head, reassembled

Sharding: batch-parallel, one batch element per NeuronCore (8 cores).
Host-side prep (numpy, layout + fp16 cast): transpose weights to [di, do]
and build ctxT = concat(hidden, context).transpose -> [D, NK] per batch so
the contraction dim lands on SBUF partitions.

Structure: V projection first, then one fused loop over head pairs that
computes the K/Q projection slices for that pair and immediately runs
attention on them.  Scores for the head pair run as row-tiled matmuls at
partition offsets 0/64.  Softmax denominators come for free from a
ones-column appended to v.  All matmuls run in fp16 with fp32 PSUM
accumulation.  The attention output stays in [dh, nq] orientation: the
denominator row is reciprocal'd and partition-broadcast into a single DVE
multiply, written as fp16 [H, DH, NQ] and un-permuted on the host -- this
removes the PE transposes and PSUM->SBUF staging copies of the earlier
version.  Projection loops interleave the output chunks di-innermost so
consecutive matmuls share LDWEIGHTS.  ctxT is double-buffered so the next
iteration's input DMA overlaps compute (relevant for the repeat-loop
timing harness; single-shot unaffected).

Biases are all-zero for this problem spec and are ignored.
"""
import numpy as np

import concourse.bass as bass
import concourse.mybir as mybir
import concourse.tile as tile
from concourse import bacc
from concourse.bass import ds, ts
from concourse.bass_utils import run_bass_kernel_spmd

N_CORES = 8
P = 128
D = 1024          # model dim
NQ = 1024         # query length (hidden)
NK = 1280         # key/value length (hidden + context)
H = 16            # heads
DH = 64           # head dim
DT = D // P       # 8 contraction tiles
NKT = NK // P     # 10 nk tiles
SCALE = 1.0 / 8.0  # 1/sqrt(DH)
F32 = mybir.dt.float32
F16 = mybir.dt.float16
NQC = 512         # nq chunk for attention
NCH = NQ // NQC   # 2 chunks


def emit(nc, tc, ctx_d, wq_d, wk_d, wv_d, out_d, repeat=1):
    with (
        tc.tile_pool(name="persist", bufs=1) as persist,
        tc.tile_pool(name="ctxp", bufs=2) as ctxp,
        tc.tile_pool(name="wp", bufs=16) as wp,
        tc.tile_pool(name="kqp", bufs=2) as kqp,
        tc.tile_pool(name="p2", bufs=4) as p2,
        tc.tile_pool(name="stg", bufs=4) as stg,
        tc.tile_pool(name="psp", bufs=2, space="PSUM") as psp,
        tc.tile_pool(name="pss", bufs=2, space="PSUM") as pss,
        tc.tile_pool(name="pso", bufs=2, space="PSUM") as pso,
    ):
        v = persist.tile([P, NKT, H, DH + 1], F16, tag="v")
        nc.vector.memset(v[:, :, :, DH:DH + 1], 1.0)
        if repeat == 1:
            _emit_iter(nc, tc, ctxp, wp, kqp, p2, stg, psp, pss, pso,
                       v, ctx_d, wq_d, wk_d, wv_d, out_d)
        else:
            # hardware loop: used only for wall-clock timing builds
            with tc.For_i(0, repeat, 1):
                _emit_iter(nc, tc, ctxp, wp, kqp, p2, stg, psp, pss, pso,
                           v, ctx_d, wq_d, wk_d, wv_d, out_d)


def _emit_iter(nc, tc, ctxp, wp, kqp, p2, stg, psp, pss, pso,
               v, ctx_d, wq_d, wk_d, wv_d, out_d):
    ctxT = ctxp.tile([P, DT, NK], F16, tag="ctxT")

    def load_w(w_d, name, eng):
        tiles = []
        for t in range(DT):
            wt = wp.tile([P, D], F16, tag="w", name=f"{name}_{t}")
            eng.dma_start(wt[:], w_d[ts(t, P), :])
            tiles.append(wt)
        return tiles

    # DMA order: ctxT and wv interleaved per di feed the V projection that
    # runs first (di-innermost, so each di needs the full wv tile).  The
    # sync (SP) queue carries ONLY input DMAs so the next repeat-iteration's
    # prefetch is never stuck behind steady-state store traffic.
    wv = []
    for t in range(DT):
        nc.sync.dma_start(ctxT[:, t, :], ctx_d[ts(t, P), :])
        wt = wp.tile([P, D], F16, tag="w", name=f"wv_{t}")
        nc.sync.dma_start(wt[:], wv_d[ts(t, P), :])
        wv.append(wt)
    wk = load_w(wk_d, "wk", nc.sync)
    wq = load_w(wq_d, "wq", nc.sync)

    # ---- V projection: v[nk, do] = sum_di ctxT[di, nk] * WvT[di, do] ----
    # di-innermost over both do-halves: consecutive matmuls share weights.
    for m in range(NKT):
        ps0 = psp.tile([P, 512], F32, tag="ps", name=f"vps0_{m}")
        ps1 = psp.tile([P, 512], F32, tag="ps", name=f"vps1_{m}")
        for di in range(DT):
            nc.tensor.matmul(
                ps0[:], ctxT[:, di, ts(m, P)], wv[di][:, 0:512],
                start=(di == 0), stop=(di == DT - 1),
            )
            nc.tensor.matmul(
                ps1[:], ctxT[:, di, ts(m, P)], wv[di][:, 512:1024],
                start=(di == 0), stop=(di == DT - 1),
            )
        for g, ps in ((0, ps0), (1, ps1)):
            nc.vector.tensor_copy(
                v[:, m, ds(g * 8, 8), 0:DH],
                ps[:].rearrange("p (h d) -> p h d", h=8),
            )

    # ---- fused loop over head pairs ----
    for hp in range(H // 2):
        pair = (2 * hp, 2 * hp + 1)
        # K slice for this pair: kT[do=hp-tile, nk]; nk 0:1024 di-interleaved
        kT = kqp.tile([P, NK], F16, tag="kT", name=f"kT_{hp}")
        kp0 = psp.tile([P, 512], F32, tag="ps", name=f"kps0_{hp}")
        kp1 = psp.tile([P, 512], F32, tag="ps", name=f"kps1_{hp}")
        for di in range(DT):
            nc.tensor.matmul(
                kp0[:], wk[di][:, ts(hp, P)], ctxT[:, di, 0:512],
                start=(di == 0), stop=(di == DT - 1),
            )
            nc.tensor.matmul(
                kp1[:], wk[di][:, ts(hp, P)], ctxT[:, di, 512:1024],
                start=(di == 0), stop=(di == DT - 1),
            )
        nc.vector.tensor_copy(kT[:, 0:512], kp0[:])
        nc.vector.tensor_copy(kT[:, 512:1024], kp1[:])
        kp2 = psp.tile([P, 512], F32, tag="ps", name=f"kps2_{hp}")
        for di in range(DT):
            nc.tensor.matmul(
                kp2[:, 0:256], wk[di][:, ts(hp, P)], ctxT[:, di, 1024:1280],
                start=(di == 0), stop=(di == DT - 1),
            )
        nc.vector.tensor_copy(kT[:, 1024:1280], kp2[:, 0:256])
        # Q slice for this pair: qT[do=hp-tile, nq], chunks di-interleaved
        qT = kqp.tile([P, NQ], F16, tag="qT", name=f"qT_{hp}")
        qp0 = psp.tile([P, 512], F32, tag="ps", name=f"qps0_{hp}")
        qp1 = psp.tile([P, 512], F32, tag="ps", name=f"qps1_{hp}")
        for di in range(DT):
            nc.tensor.matmul(
                qp0[:], wq[di][:, ts(hp, P)], ctxT[:, di, 0:512],
                start=(di == 0), stop=(di == DT - 1),
            )
            nc.tensor.matmul(
                qp1[:], wq[di][:, ts(hp, P)], ctxT[:, di, 512:1024],
                start=(di == 0), stop=(di == DT - 1),
            )
        nc.vector.tensor_copy(qT[:, 0:512], qp0[:])
        nc.vector.tensor_copy(qT[:, 512:1024], qp1[:])

        for c in range(NCH):
            et = {
                h: p2.tile([P, NKT, NQC], F16, tag="expT", name=f"expT_{h}")
                for h in pair
            }
            # scoresT[nk, nq]: head pair at partition offsets 0/64 emitted
            # interleaved; 2 nk-tiles share a 2-bank psum tile so exp runs
            # as one big ACT instruction.
            for g in range(NKT // 2):
                pp = {
                    h: pss.tile([P, 2, NQC], F32, tag="pss", name=f"pss_{h}")
                    for h in pair
                }
                for tt in range(2):
                    for h in pair:
                        o = 64 * (h % 2)
                        nc.tensor.matmul(
                            pp[h][:, tt, :],
                            kT[o:o + DH, ts(2 * g + tt, P)],
                            qT[o:o + DH, ds(c * NQC, NQC)],
                            start=True,
                            stop=True,
                        )
                for h in pair:
                    nc.scalar.activation(
                        et[h][:, ds(2 * g, 2), :], pp[h][:, :, :],
                        mybir.ActivationFunctionType.Exp,
                        scale=SCALE,
                    )
            # outT_aug[65, nq] = sum_nk v_aug[nk, 65] * expT[nk, nq];
            # row 64 is the softmax denominator: reciprocal (lane-aligned),
            # SBUF DMA hop to partition 0, partition-broadcast, one DVE
            # multiply; result stays [dh, nq] (host un-permutes).  The two
            # heads share one hop/broadcast/store to halve small-DMA count.
            otn = stg.tile([DH, 2, NQC], F16, tag="otn")
            for h in pair:
                po = pso.tile([DH + 1, NQC], F32, tag="pso",
                              name=f"po_{h}")
                for t in range(NKT):
                    nc.tensor.matmul(
                        po[:],
                        v[:, t, h, :],
                        et[h][:, t, :],
                        start=(t == 0),
                        stop=(t == NKT - 1),
                    )
                rc64 = stg.tile([DH + 1, NQC], F32, tag="rc64",
                                name=f"rc64_{h}")
                nc.vector.reciprocal(rc64[DH:DH + 1, :], po[DH:DH + 1, :])
                rc0 = stg.tile([1, NQC], F32, tag="rc0", name=f"rc0_{h}")
                nc.gpsimd.dma_start(rc0[0:1, :], rc64[DH:DH + 1, :])
                rb = stg.tile([DH, NQC], F32, tag="rb", name=f"rb_{h}")
                nc.gpsimd.partition_broadcast(rb[:], rc0[0:1, :],
                                              channels=DH)
                nc.vector.scalar_tensor_tensor(
                    otn[:, h % 2, :], po[0:DH, :], 1.0, rb[:],
                    op0=mybir.AluOpType.bypass,
                    op1=mybir.AluOpType.mult,
                )
            nc.gpsimd.dma_start(
                out_d[ds(2 * hp, 2), :, ds(c * NQC, NQC)].rearrange(
                    "h d q -> d h q"
                ),
                otn[:],
            )


_CACHE = {}


def build(repeat=1):
    key = repeat
    if key in _CACHE:
        return _CACHE[key]
    nc = bacc.Bacc("TRN2", target_bir_lowering=False, debug=False,
                   num_devices=N_CORES)
    ctx_d = nc.dram_tensor("ctxT", [D, NK], F16, kind="ExternalInput")
    wq_d = nc.dram_tensor("wqT", [D, D], F16, kind="ExternalInput")
    wk_d = nc.dram_tensor("wkT", [D, D], F16, kind="ExternalInput")
    wv_d = nc.dram_tensor("wvT", [D, D], F16, kind="ExternalInput")
    out_d = nc.dram_tensor("out", [H, DH, NQ], F16, kind="ExternalOutput")
    with tile.TileContext(nc) as tc:
        emit(nc, tc, ctx_d, wq_d, wk_d, wv_d, out_d, repeat=repeat)
    nc.compile()
    _CACHE[key] = (nc, ctx_d, wq_d, wk_d, wv_d, out_d)
    return _CACHE[key]


def make_in_maps(hidden_states, context_states, Wq, Wk, Wv):
    ctxT = np.ascontiguousarray(
        np.concatenate([hidden_states, context_states], axis=1).transpose(0, 2, 1)
    ).astype(np.float16)
    wqT = np.ascontiguousarray(np.asarray(Wq).T).astype(np.float16)
    wkT = np.ascontiguousarray(np.asarray(Wk).T).astype(np.float16)
    wvT = np.ascontiguousarray(np.asarray(Wv).T).astype(np.float16)
    return [
        {"ctxT": ctxT[b], "wqT": wqT, "wkT": wkT, "wvT": wvT}
        for b in range(N_CORES)
    ]


def kernel(hidden_states, context_states, Wq, bq, Wk, bk, Wv, bv):
    # bq/bk/bv are zeros per the problem spec; not applied.
    nc = build(repeat=1)[0]
    in_maps = make_in_maps(hidden_states, context_states, Wq, Wk, Wv)
    res = run_bass_kernel_spmd(nc, in_maps, core_ids=list(range(N_CORES)))
    # device writes [H, DH, NQ] fp16; un-permute to [NQ, D] on host
    return np.stack(
        [
            np.ascontiguousarray(
                res.results[b]["out"].transpose(2, 0, 1)
            ).reshape(NQ, H * DH).astype(np.float32)
            for b in range(N_CORES)
        ],
        axis=0,
    )


# revision 10
# speedup vs baseline: 1.0394x; 1.0394x over previous
"""Trainium2 Bass kernel for BaseViTSelfAttention (cross/self attention, 16 heads).

Computation (per batch element b):
    q = hidden @ Wq.T            [1024, 1024]
    ctx = concat(hidden, context)  [1280, 1024]
    k = ctx @ Wk.T; v = ctx @ Wv.T
    out = softmax(q_h @ k_h.T / 8) @ v_h   per 64-dim head, reassembled

Sharding: batch-parallel, one batch element per NeuronCore (8 cores).
Host-side prep (numpy, layout + fp16 cast): transpose weights to [di, do]
and build ctxT = concat(hidden, context).transpose -> [D, NK] per batch so
the contraction dim lands on SBUF partitions.

Structure: V projection first, then one fused loop over head pairs that
computes the K/Q projection slices for that pair and immediately runs
attention on them.  Scores for the head pair run as row-tiled matmuls at
partition offsets 0/64.  Softmax denominators come for free from a
ones-column appended to v.  All matmuls run in fp16 with fp32 PSUM
accumulation.  The attention output stays in [dh, nq] orientation: the
denominator row is reciprocal'd and partition-broadcast into a single DVE
multiply, written as fp16 [H, DH, NQ] and un-permuted on the host -- this
removes the PE transposes and PSUM->SBUF staging copies of the earlier
version.  Projection loops interleave the output chunks di-innermost so
consecutive matmuls share LDWEIGHTS.  ctxT is double-buffered so the next
iteration's input DMA overlaps compute (relevant for the repeat-loop
timing harness; single-shot unaffected).

Biases are all-zero for this problem spec and are ignored.
"""
import numpy as np

import concourse.bass as bass
import concourse.mybir as mybir
import concourse.tile as tile
from concourse import bacc
from concourse.bass import ds, ts
from concourse.bass_utils import run_bass_kernel_spmd

N_CORES = 8
P = 128
D = 1024          # model dim
NQ = 1024         # query length (hidden)
NK = 1280         # key/value length (hidden + context)
H = 16            # heads
DH = 64           # head dim
DT = D // P       # 8 contraction tiles
NKT = NK // P     # 10 nk tiles
SCALE = 1.0 / 8.0  # 1/sqrt(DH)
F32 = mybir.dt.float32
F16 = mybir.dt.float16
NQC = 512         # nq chunk for attention
NCH = NQ // NQC   # 2 chunks


def emit(nc, tc, ctx_d, wq_d, wk_d, wv_d, out_d, repeat=1):
    with (
        tc.tile_pool(name="persist", bufs=1) as persist,
        tc.tile_pool(name="ctxp", bufs=2) as ctxp,
        tc.tile_pool(name="wp", bufs=16) as wp,
        tc.tile_pool(name="kqp", bufs=2) as kqp,
        tc.tile_pool(name="p2", bufs=4) as p2,
        tc.tile_pool(name="stg", bufs=4) as stg,
        tc.tile_pool(name="psp", bufs=2, space="PSUM") as psp,
        tc.tile_pool(name="pss", bufs=2, space="PSUM") as pss,
        tc.tile_pool(name="pso", bufs=1, space="PSUM") as pso,
        tc.tile_pool(name="pst", bufs=1, space="PSUM") as pst,
    ):
        from concourse.masks import make_identity
        ident = persist.tile([P, P], F32, tag="ident")
        make_identity(nc, ident[:])
        v = persist.tile([P, NKT, H, DH + 1], F16, tag="v")
        nc.vector.memset(v[:, :, :, DH:DH + 1], 1.0)
        if repeat == 1:
            _emit_iter(nc, tc, ctxp, wp, kqp, p2, stg, psp, pss, pso, pst,
                       ident, v, ctx_d, wq_d, wk_d, wv_d, out_d)
        else:
            # hardware loop: used only for wall-clock timing builds
            with tc.For_i(0, repeat, 1):
                _emit_iter(nc, tc, ctxp, wp, kqp, p2, stg, psp, pss, pso,
                           pst, ident, v, ctx_d, wq_d, wk_d, wv_d, out_d)


def _emit_iter(nc, tc, ctxp, wp, kqp, p2, stg, psp, pss, pso, pst,
               ident, v, ctx_d, wq_d, wk_d, wv_d, out_d):
    ctxT = ctxp.tile([P, DT, NK], F16, tag="ctxT")

    def load_w(w_d, name, eng):
        tiles = []
        for t in range(DT):
            wt = wp.tile([P, D], F16, tag="w", name=f"{name}_{t}")
            eng.dma_start(wt[:], w_d[ts(t, P), :])
            tiles.append(wt)
        return tiles

    # DMA order: ctxT and wv interleaved per di feed the V projection that
    # runs first (di-innermost, so each di needs the full wv tile).  The
    # sync (SP) queue carries ONLY input DMAs so the next repeat-iteration's
    # prefetch is never stuck behind steady-state store traffic.
    wv = []
    for t in range(DT):
        nc.sync.dma_start(ctxT[:, t, :], ctx_d[ts(t, P), :])
        wt = wp.tile([P, D], F16, tag="w", name=f"wv_{t}")
        nc.sync.dma_start(wt[:], wv_d[ts(t, P), :])
        wv.append(wt)
    wk = load_w(wk_d, "wk", nc.sync)
    wq = load_w(wq_d, "wq", nc.sync)

    # ---- V projection: v[nk, do] = sum_di ctxT[di, nk] * WvT[di, do] ----
    # di-innermost over both do-halves: consecutive matmuls share weights.
    for m in range(NKT):
        ps0 = psp.tile([P, 512], F32, tag="ps", name=f"vps0_{m}")
        ps1 = psp.tile([P, 512], F32, tag="ps", name=f"vps1_{m}")
        for di in range(DT):
            nc.tensor.matmul(
                ps0[:], ctxT[:, di, ts(m, P)], wv[di][:, 0:512],
                start=(di == 0), stop=(di == DT - 1),
            )
            nc.tensor.matmul(
                ps1[:], ctxT[:, di, ts(m, P)], wv[di][:, 512:1024],
                start=(di == 0), stop=(di == DT - 1),
            )
        for g, ps in ((0, ps0), (1, ps1)):
            nc.vector.tensor_copy(
                v[:, m, ds(g * 8, 8), 0:DH],
                ps[:].rearrange("p (h d) -> p h d", h=8),
            )

    # ---- fused loop over head pairs ----
    for hp in range(H // 2):
        pair = (2 * hp, 2 * hp + 1)
        # K slice for this pair: kT[do=hp-tile, nk]; nk 0:1024 di-interleaved
        kT = kqp.tile([P, NK], F16, tag="kT", name=f"kT_{hp}")
        kp0 = psp.tile([P, 512], F32, tag="ps", name=f"kps0_{hp}")
        kp1 = psp.tile([P, 512], F32, tag="ps", name=f"kps1_{hp}")
        for di in range(DT):
            nc.tensor.matmul(
                kp0[:], wk[di][:, ts(hp, P)], ctxT[:, di, 0:512],
                start=(di == 0), stop=(di == DT - 1),
            )
            nc.tensor.matmul(
                kp1[:], wk[di][:, ts(hp, P)], ctxT[:, di, 512:1024],
                start=(di == 0), stop=(di == DT - 1),
            )
        nc.vector.tensor_copy(kT[:, 0:512], kp0[:])
        nc.vector.tensor_copy(kT[:, 512:1024], kp1[:])
        kp2 = psp.tile([P, 512], F32, tag="ps", name=f"kps2_{hp}")
        for di in range(DT):
            nc.tensor.matmul(
                kp2[:, 0:256], wk[di][:, ts(hp, P)], ctxT[:, di, 1024:1280],
                start=(di == 0), stop=(di == DT - 1),
            )
        nc.vector.tensor_copy(kT[:, 1024:1280], kp2[:, 0:256])
        # Q slice for this pair: qT[do=hp-tile, nq], chunks di-interleaved
        qT = kqp.tile([P, NQ], F16, tag="qT", name=f"qT_{hp}")
        qp0 = psp.tile([P, 512], F32, tag="ps", name=f"qps0_{hp}")
        qp1 = psp.tile([P, 512], F32, tag="ps", name=f"qps1_{hp}")
        for di in range(DT):
            nc.tensor.matmul(
                qp0[:], wq[di][:, ts(hp, P)], ctxT[:, di, 0:512],
                start=(di == 0), stop=(di == DT - 1),
            )
            nc.tensor.matmul(
                qp1[:], wq[di][:, ts(hp, P)], ctxT[:, di, 512:1024],
                start=(di == 0), stop=(di == DT - 1),
            )
        nc.vector.tensor_copy(qT[:, 0:512], qp0[:])
        nc.vector.tensor_copy(qT[:, 512:1024], qp1[:])

        for c in range(NCH):
            et = {
                h: p2.tile([P, NKT, NQC], F16, tag="expT", name=f"expT_{h}")
                for h in pair
            }
            # scoresT[nk, nq]: head pair at partition offsets 0/64 emitted
            # interleaved; 2 nk-tiles share a 2-bank psum tile so exp runs
            # as one big ACT instruction.
            for g in range(NKT // 2):
                pp = {
                    h: pss.tile([P, 2, NQC], F32, tag="pss", name=f"pss_{h}")
                    for h in pair
                }
                for tt in range(2):
                    for h in pair:
                        o = 64 * (h % 2)
                        nc.tensor.matmul(
                            pp[h][:, tt, :],
                            kT[o:o + DH, ts(2 * g + tt, P)],
                            qT[o:o + DH, ds(c * NQC, NQC)],
                            start=True,
                            stop=True,
                        )
                for h in pair:
                    nc.scalar.activation(
                        et[h][:, ds(2 * g, 2), :], pp[h][:, :, :],
                        mybir.ActivationFunctionType.Exp,
                        scale=SCALE,
                    )
            # outT_aug[65, nq] = sum_nk v_aug[nk, 65] * expT[nk, nq]
            otp = [
                stg.tile([P, 2, DH], F32, tag="outstg", name=f"otp_{c}_{j}")
                for j in range(NQC // P)
            ]
            for h in pair:
                po = pso.tile([DH + 1, NQC], F32, tag="pso")
                for t in range(NKT):
                    nc.tensor.matmul(
                        po[:],
                        v[:, t, h, :],
                        et[h][:, t, :],
                        start=(t == 0),
                        stop=(t == NKT - 1),
                    )
                st = stg.tile([DH + 1, NQC], F32, tag="stage")
                nc.vector.tensor_copy(st[:], po[:])
                for j in range(NQC // P):
                    pt = pst.tile([P, DH + 1], F32, tag="pst")
                    nc.tensor.transpose(
                        pt[:], st[:, ts(j, P)], ident[:DH + 1, :DH + 1]
                    )
                    rc = stg.tile([P, 1], F32, tag="recip")
                    nc.vector.reciprocal(rc[:], pt[:, DH:DH + 1])
                    nc.vector.tensor_scalar_mul(
                        otp[j][:, h % 2, :], pt[:, 0:DH], rc[:]
                    )
            for j in range(NQC // P):
                nt = c * (NQC // P) + j
                eng = nc.gpsimd
                # out_d layout [H/2, NQ, 2*DH]: one contiguous 64KB block
                eng.dma_start(out_d[hp, ts(nt, P), :], otp[j][:])


_CACHE = {}


def build(repeat=1):
    key = repeat
    if key in _CACHE:
        return _CACHE[key]
    nc = bacc.Bacc("TRN2", target_bir_lowering=False, debug=False,
                   num_devices=N_CORES)
    ctx_d = nc.dram_tensor("ctxT", [D, NK], F16, kind="ExternalInput")
    wq_d = nc.dram_tensor("wqT", [D, D], F16, kind="ExternalInput")
    wk_d = nc.dram_tensor("wkT", [D, D], F16, kind="ExternalInput")
    wv_d = nc.dram_tensor("wvT", [D, D], F16, kind="ExternalInput")
    out_d = nc.dram_tensor("out", [H // 2, NQ, 2 * DH], F32,
                           kind="ExternalOutput")
    with tile.TileContext(nc) as tc:
        emit(nc, tc, ctx_d, wq_d, wk_d, wv_d, out_d, repeat=repeat)
    nc.compile()
    _CACHE[key] = (nc, ctx_d, wq_d, wk_d, wv_d, out_d)
    return _CACHE[key]


def make_in_maps(hidden_states, context_states, Wq, Wk, Wv):
    ctxT = np.ascontiguousarray(
        np.concatenate([hidden_states, context_states], axis=1).transpose(0, 2, 1)
    ).astype(np.float16)
    wqT = np.ascontiguousarray(np.asarray(Wq).T).astype(np.float16)
    wkT = np.ascontiguousarray(np.asarray(Wk).T).astype(np.float16)
    wvT = np.ascontiguousarray(np.asarray(Wv).T).astype(np.float16)
    return [
        {"ctxT": ctxT[b], "wqT": wqT, "wkT": wkT, "wvT": wvT}
        for b in range(N_CORES)
    ]


def kernel(hidden_states, context_states, Wq, bq, Wk, bk, Wv, bv):
    # bq/bk/bv are zeros per the problem spec; not applied.
    nc = build(repeat=1)[0]
    in_maps = make_in_maps(hidden_states, context_states, Wq, Wk, Wv)
    res = run_bass_kernel_spmd(nc, in_maps, core_ids=list(range(N_CORES)))
    # device writes [H/2, NQ, 2*DH]; un-permute to [NQ, D] on host
    return np.stack(
        [
            res.results[b]["out"].transpose(1, 0, 2).reshape(NQ, H * DH)
            for b in range(N_CORES)
        ],
        axis=0,
    )


# revision 11
# speedup vs baseline: 1.7056x; 1.6409x over previous
"""Trainium2 Bass kernel for BaseViTSelfAttention (cross/self attention, 16 heads).

Computation (per batch element b):
    q = hidden @ Wq.T            [1024, 1024]
    ctx = concat(hidden, context)  [1280, 1024]
    k = ctx @ Wk.T; v = ctx @ Wv.T
    out = softmax(q_h @ k_h.T / 8) @ v_h   per 64-dim head, reassembled

Sharding: batch-parallel, one batch element per NeuronCore (8 cores).
Host-side prep (numpy, layout + fp16 cast): transpose weights to [di, do]
and build ctxT = concat(hidden, context).transpose -> [D, NK] per batch so
the contraction dim lands on SBUF partitions.

Structure: V projection first, then one fused loop over head pairs that
computes the K/Q projection slices for that pair and immediately runs
attention on them.  Scores for the head pair run as row-tiled matmuls at
partition offsets 0/64.  Softmax denominators come for free from a
ones-column appended to v.  All matmuls run in fp16 with fp32 PSUM
accumulation.  The attention output stays in [dh, nq] orientation: the
denominator row is reciprocal'd and partition-broadcast into a single DVE
multiply, written as fp16 [H, DH, NQ] and un-permuted on the host -- this
removes the PE transposes and PSUM->SBUF staging copies of the earlier
version.  Projection loops interleave the output chunks di-innermost so
consecutive matmuls share LDWEIGHTS.  ctxT is double-buffered so the next
iteration's input DMA overlaps compute (relevant for the repeat-loop
timing harness; single-shot unaffected).

Biases are all-zero for this problem spec and are ignored.
"""
import numpy as np

import concourse.bass as bass
import concourse.mybir as mybir
import concourse.tile as tile
from concourse import bacc
from concourse.bass import ds, ts
from concourse.bass_utils import run_bass_kernel_spmd

N_CORES = 8
P = 128
D = 1024          # model dim
NQ = 1024         # query length (hidden)
NK = 1280         # key/value length (hidden + context)
H = 16            # heads
DH = 64           # head dim
DT = D // P       # 8 contraction tiles
NKT = NK // P     # 10 nk tiles
SCALE = 1.0 / 8.0  # 1/sqrt(DH)
F32 = mybir.dt.float32
F16 = mybir.dt.float16
NQC = 512         # nq chunk for attention
NCH = NQ // NQC   # 2 chunks


def emit(nc, tc, ctx_d, wq_d, wk_d, wv_d, out_d, repeat=1):
    with (
        tc.tile_pool(name="persist", bufs=1) as persist,
        tc.tile_pool(name="ctxp", bufs=2) as ctxp,
        tc.tile_pool(name="wp", bufs=16) as wp,
        tc.tile_pool(name="kqp", bufs=2) as kqp,
        tc.tile_pool(name="p2", bufs=4) as p2,
        tc.tile_pool(name="stg", bufs=4) as stg,
        tc.tile_pool(name="psp", bufs=2, space="PSUM") as psp,
        tc.tile_pool(name="pss", bufs=2, space="PSUM") as pss,
        tc.tile_pool(name="pso", bufs=1, space="PSUM") as pso,
        tc.tile_pool(name="pst", bufs=1, space="PSUM") as pst,
    ):
        from concourse.masks import make_identity
        ident = persist.tile([P, P], F32, tag="ident")
        make_identity(nc, ident[:])
        if repeat == 1:
            _emit_iter(nc, tc, persist, ctxp, wp, kqp, p2, stg, psp, pss,
                       pso, pst, ident, ctx_d, wq_d, wk_d, wv_d, out_d)
        else:
            # hardware loop: used only for wall-clock timing builds
            with tc.For_i(0, repeat, 1):
                _emit_iter(nc, tc, persist, ctxp, wp, kqp, p2, stg, psp, pss,
                           pso, pst, ident, ctx_d, wq_d, wk_d, wv_d, out_d)


def _emit_iter(nc, tc, persist, ctxp, wp, kqp, p2, stg, psp, pss, pso, pst,
               ident, ctx_d, wq_d, wk_d, wv_d, out_d):
    v = persist.tile([P, NKT, H, DH + 1], F16, tag="v")
    nc.vector.memset(v[:, :, :, DH:DH + 1], 1.0)
    ctxT = ctxp.tile([P, DT, NK], F16, tag="ctxT")

    def load_w(w_d, name, eng):
        tiles = []
        for t in range(DT):
            wt = wp.tile([P, D], F16, tag="w", name=f"{name}_{t}")
            eng.dma_start(wt[:], w_d[ts(t, P), :])
            tiles.append(wt)
        return tiles

    # DMA order: ctxT and wv interleaved per di feed the V projection that
    # runs first (di-innermost, so each di needs the full wv tile).  The
    # sync (SP) queue carries ONLY input DMAs so the next repeat-iteration's
    # prefetch is never stuck behind steady-state store traffic.
    wv = []
    for t in range(DT):
        nc.sync.dma_start(ctxT[:, t, :], ctx_d[ts(t, P), :])
        wt = wp.tile([P, D], F16, tag="w", name=f"wv_{t}")
        nc.sync.dma_start(wt[:], wv_d[ts(t, P), :])
        wv.append(wt)
    wk = load_w(wk_d, "wk", nc.sync)
    wq = load_w(wq_d, "wq", nc.sync)

    # ---- V projection: v[nk, do] = sum_di ctxT[di, nk] * WvT[di, do] ----
    # di-innermost over both do-halves: consecutive matmuls share weights.
    for m in range(NKT):
        ps0 = psp.tile([P, 512], F32, tag="ps", name=f"vps0_{m}")
        ps1 = psp.tile([P, 512], F32, tag="ps", name=f"vps1_{m}")
        for di in range(DT):
            nc.tensor.matmul(
                ps0[:], ctxT[:, di, ts(m, P)], wv[di][:, 0:512],
                start=(di == 0), stop=(di == DT - 1),
            )
            nc.tensor.matmul(
                ps1[:], ctxT[:, di, ts(m, P)], wv[di][:, 512:1024],
                start=(di == 0), stop=(di == DT - 1),
            )
        for g, ps in ((0, ps0), (1, ps1)):
            nc.vector.tensor_copy(
                v[:, m, ds(g * 8, 8), 0:DH],
                ps[:].rearrange("p (h d) -> p h d", h=8),
            )

    # ---- fused loop over head pairs ----
    for hp in range(H // 2):
        pair = (2 * hp, 2 * hp + 1)
        # K slice for this pair: kT[do=hp-tile, nk]; nk 0:1024 di-interleaved
        kT = kqp.tile([P, NK], F16, tag="kT", name=f"kT_{hp}")
        kp0 = psp.tile([P, 512], F32, tag="ps", name=f"kps0_{hp}")
        kp1 = psp.tile([P, 512], F32, tag="ps", name=f"kps1_{hp}")
        for di in range(DT):
            nc.tensor.matmul(
                kp0[:], wk[di][:, ts(hp, P)], ctxT[:, di, 0:512],
                start=(di == 0), stop=(di == DT - 1),
            )
            nc.tensor.matmul(
                kp1[:], wk[di][:, ts(hp, P)], ctxT[:, di, 512:1024],
                start=(di == 0), stop=(di == DT - 1),
            )
        nc.vector.tensor_copy(kT[:, 0:512], kp0[:])
        nc.vector.tensor_copy(kT[:, 512:1024], kp1[:])
        kp2 = psp.tile([P, 512], F32, tag="ps", name=f"kps2_{hp}")
        for di in range(DT):
            nc.tensor.matmul(
                kp2[:, 0:256], wk[di][:, ts(hp, P)], ctxT[:, di, 1024:1280],
                start=(di == 0), stop=(di == DT - 1),
            )
        nc.vector.tensor_copy(kT[:, 1024:1280], kp2[:, 0:256])
        # Q slice for this pair: qT[do=hp-tile, nq], chunks di-interleaved
        qT = kqp.tile([P, NQ], F16, tag="qT", name=f"qT_{hp}")
        qp0 = psp.tile([P, 512], F32, tag="ps", name=f"qps0_{hp}")
        qp1 = psp.tile([P, 512], F32, tag="ps", name=f"qps1_{hp}")
        for di in range(DT):
            nc.tensor.matmul(
                qp0[:], wq[di][:, ts(hp, P)], ctxT[:, di, 0:512],
                start=(di == 0), stop=(di == DT - 1),
            )
            nc.tensor.matmul(
                qp1[:], wq[di][:, ts(hp, P)], ctxT[:, di, 512:1024],
                start=(di == 0), stop=(di == DT - 1),
            )
        nc.vector.tensor_copy(qT[:, 0:512], qp0[:])
        nc.vector.tensor_copy(qT[:, 512:1024], qp1[:])

        for c in range(NCH):
            et = {
                h: p2.tile([P, NKT, NQC], F16, tag="expT", name=f"expT_{h}")
                for h in pair
            }
            # scoresT[nk, nq]: head pair at partition offsets 0/64 emitted
            # interleaved; 2 nk-tiles share a 2-bank psum tile so exp runs
            # as one big ACT instruction.
            for g in range(NKT // 2):
                pp = {
                    h: pss.tile([P, 2, NQC], F32, tag="pss", name=f"pss_{h}")
                    for h in pair
                }
                for tt in range(2):
                    for h in pair:
                        o = 64 * (h % 2)
                        nc.tensor.matmul(
                            pp[h][:, tt, :],
                            kT[o:o + DH, ts(2 * g + tt, P)],
                            qT[o:o + DH, ds(c * NQC, NQC)],
                            start=True,
                            stop=True,
                        )
                for h in pair:
                    nc.scalar.activation(
                        et[h][:, ds(2 * g, 2), :], pp[h][:, :, :],
                        mybir.ActivationFunctionType.Exp,
                        scale=SCALE,
                    )
            # outT_aug[65, nq] = sum_nk v_aug[nk, 65] * expT[nk, nq]
            otp = [
                stg.tile([P, 2, DH], F32, tag="outstg", name=f"otp_{c}_{j}")
                for j in range(NQC // P)
            ]
            for h in pair:
                po = pso.tile([DH + 1, NQC], F32, tag="pso")
                for t in range(NKT):
                    nc.tensor.matmul(
                        po[:],
                        v[:, t, h, :],
                        et[h][:, t, :],
                        start=(t == 0),
                        stop=(t == NKT - 1),
                    )
                st = stg.tile([DH + 1, NQC], F32, tag="stage")
                nc.vector.tensor_copy(st[:], po[:])
                for j in range(NQC // P):
                    pt = pst.tile([P, DH + 1], F32, tag="pst")
                    nc.tensor.transpose(
                        pt[:], st[:, ts(j, P)], ident[:DH + 1, :DH + 1]
                    )
                    rc = stg.tile([P, 1], F32, tag="recip")
                    nc.vector.reciprocal(rc[:], pt[:, DH:DH + 1])
                    nc.vector.tensor_scalar_mul(
                        otp[j][:, h % 2, :], pt[:, 0:DH], rc[:]
                    )
            for j in range(NQC // P):
                nt = c * (NQC // P) + j
                eng = nc.gpsimd
                # out_d layout [H/2, NQ, 2*DH]: one contiguous 64KB block
                eng.dma_start(out_d[hp, ts(nt, P), :], otp[j][:])


_CACHE = {}


def build(repeat=1):
    key = repeat
    if key in _CACHE:
        return _CACHE[key]
    nc = bacc.Bacc("TRN2", target_bir_lowering=False, debug=False,
                   num_devices=N_CORES)
    ctx_d = nc.dram_tensor("ctxT", [D, NK], F16, kind="ExternalInput")
    wq_d = nc.dram_tensor("wqT", [D, D], F16, kind="ExternalInput")
    wk_d = nc.dram_tensor("wkT", [D, D], F16, kind="ExternalInput")
    wv_d = nc.dram_tensor("wvT", [D, D], F16, kind="ExternalInput")
    out_d = nc.dram_tensor("out", [H // 2, NQ, 2 * DH], F32,
                           kind="ExternalOutput")
    with tile.TileContext(nc) as tc:
        emit(nc, tc, ctx_d, wq_d, wk_d, wv_d, out_d, repeat=repeat)
    nc.compile()
    _CACHE[key] = (nc, ctx_d, wq_d, wk_d, wv_d, out_d)
    return _CACHE[key]


def make_in_maps(hidden_states, context_states, Wq, Wk, Wv):
    ctxT = np.ascontiguousarray(
        np.concatenate([hidden_states, context_states], axis=1).transpose(0, 2, 1)
    ).astype(np.float16)
    wqT = np.ascontiguousarray(np.asarray(Wq).T).astype(np.float16)
    wkT = np.ascontiguousarray(np.asarray(Wk).T).astype(np.float16)
    wvT = np.ascontiguousarray(np.asarray(Wv).T).astype(np.float16)
    return [
        {"ctxT": ctxT[b], "wqT": wqT, "wkT": wkT, "wvT": wvT}
        for b in range(N_CORES)
    ]


def kernel(hidden_states, context_states, Wq, bq, Wk, bk, Wv, bv):
    # bq/bk/bv are zeros per the problem spec; not applied.
    nc = build(repeat=1)[0]
    in_maps = make_in_maps(hidden_states, context_states, Wq, Wk, Wv)
    res = run_bass_kernel_spmd(nc, in_maps, core_ids=list(range(N_CORES)))
    # device writes [H/2, NQ, 2*DH]; un-permute to [NQ, D] on host
    return np.stack(
        [
            res.results[b]["out"].transpose(1, 0, 2).reshape(NQ, H * DH)
            for b in range(N_CORES)
        ],
        axis=0,
    )
